# revision 6
# baseline (speedup 1.0000x reference)
"""Causal self-attention Bass kernel for 8 trn2 NeuronCores.

Problem: B=4, T=2048, D=1024, H=16 causal self-attention (qkv proj + attn + out proj).

Sharding: core c = 2*b + g handles batch b (=c//2) and head-group g (=c%2, 8 heads).

Structure (J-pipelined, chunked collective):
  - x is transposed + bf16-cast on the HOST: xT [D, T] arrives ready for matmuls.
  - V projection per t-quarter: vv[i] [keys=128, 4mp x (65A|65B)] bf16, ones col
    at slot 64 of each 65-block (softmax denominator via the AV matmul).
  - QK projection per T-quarter n: qkT[m] [128 dims, T] bf16 (+bias).
  - Attention per query block J (512 q) x head pair mp: transposed-score flash
    loop over key blocks j<=diag; exp on ACT -> at bf16; causal masks on DVE;
    AV accumulates [65, 512] psum (row 64 = denominator).
  - Normalization per (mp, J): reciprocal_approx_fast on the denominator row,
    partition_broadcast, muls; B-half is normalized pre-shift then DMA'd to
    partitions 64:128.
  - Output projection per J -> rs_in[J] bf16 [512, 1024]; pairwise
    ReduceScatter per J (overlaps attention of J+1); even core gets rows
    [0:256) of each 512-row chunk, odd gets [256:512).
Host reassembles interleaved 256-row chunks per batch.
"""

from contextlib import ExitStack

import ml_dtypes
import numpy as np

import concourse.bass as bass
import concourse.mybir as mybir
import concourse.tile as tile
from concourse import bacc
from concourse.bass_utils import run_bass_kernel_spmd

B, T, D, H = 4, 2048, 1024, 16
HD = D // H  # 64
NCORES = 8
P = 128
f32 = mybir.dt.float32
f32r = mybir.dt.float32r
bf16 = mybir.dt.bfloat16
EXP = mybir.ActivationFunctionType.Exp

_CACHE = {}
LAST_RESULTS = None
_DEBUG_SINK = None


def _dbg(nc, name, ap):
    if _DEBUG_SINK is not None and name in _DEBUG_SINK:
        nc.sync.dma_start(_DEBUG_SINK[name].ap(), ap)


def _emit(nc, tc, xt_d, wqk_d, wv_d, bqk_d, wproj_d, beta_d, out_d):
    with ExitStack() as ctx:
        # ---------------- constants / persistent tiles ----------------
        const = ctx.enter_context(tc.tile_pool(name="const", bufs=1))
        mask_tri = const.tile([P, P], bf16, tag="mask_tri")
        nc.gpsimd.memset(mask_tri[:], 1.0)
        nc.gpsimd.affine_select(
            out=mask_tri[:], in_=mask_tri[:],
            compare_op=mybir.AluOpType.is_ge, fill=0.0,
            base=0, pattern=[[1, P]], channel_multiplier=-1,
        )
        zeros384 = const.tile([P, 384], bf16, tag="zeros384")
        nc.vector.memset(zeros384[:], 0.0)
        bq = [const.tile([P, 1], f32, tag=f"bq{m}", name=f"bq{m}") for m in range(8)]
        beta_b = const.tile([P, D], bf16, tag="beta_b")

        # persistent activations
        xt_pool = ctx.enter_context(tc.tile_pool(name="xt", bufs=1))
        xT = [xt_pool.tile([P, T], bf16, tag=f"xT{k}", name=f"xT{k}") for k in range(8)]
        qkt_pool = ctx.enter_context(tc.tile_pool(name="qkt", bufs=1))
        qkT = [qkt_pool.tile([P, T], bf16, tag=f"qkT{m}", name=f"qkT{m}") for m in range(8)]
        vv_pool = ctx.enter_context(tc.tile_pool(name="vv", bufs=1))
        vv = [vv_pool.tile([P, 520], bf16, tag=f"vv{i}", name=f"vv{i}") for i in range(16)]

        # weights
        wp = ctx.enter_context(tc.tile_pool(name="wts", bufs=1))
        wqk_t = [wp.tile([P, 1024], bf16, tag=f"wqk{k}", name=f"wqk{k}") for k in range(8)]
        wv_t = [wp.tile([P, 512], bf16, tag=f"wv{k}", name=f"wv{k}") for k in range(8)]
        wproj_t = [wp.tile([P, D], bf16, tag=f"wp{hp}", name=f"wp{hp}") for hp in range(4)]

        # ones columns of vv (denominator trick): col 64 of each 65-block
        ones8 = const.tile([P, 8], bf16, tag="ones8")
        nc.vector.memset(ones8[:], 1.0)
        ones_src = ones8[:].rearrange("p (mp h one) -> p mp h one", mp=4, h=2)
        for i in range(16):
            dst = vv[i][:].rearrange("p (mp h d) -> p mp h d", mp=4, h=2)
            nc.vector.tensor_copy(dst[:, :, :, 64:65], ones_src[:, :, :, :])

        # working pools
        onp = ctx.enter_context(tc.tile_pool(name="outn", bufs=2))
        atp = ctx.enter_context(tc.tile_pool(name="atp", bufs=3))
        nrm = ctx.enter_context(tc.tile_pool(name="nrm", bufs=2))
        finp = ctx.enter_context(tc.tile_pool(name="finp", bufs=3))

        spp = ctx.enter_context(tc.tile_pool(name="spp", bufs=2, space="PSUM"))
        stps = ctx.enter_context(tc.tile_pool(name="stps", bufs=2, space="PSUM"))
        oups = ctx.enter_context(tc.tile_pool(name="oups", bufs=1, space="PSUM"))

        dram = ctx.enter_context(tc.tile_pool(name="dram", bufs=1, space="DRAM"))
        rs_in = [dram.tile([512, D], bf16, tag=f"rsi{J}", name=f"rsi{J}") for J in range(4)]
        rs_out = [dram.tile([256, D], bf16, tag=f"rso{J}", name=f"rso{J}") for J in range(4)]

        # ---------------- DMAs: quarter 0 + weights first ----------------
        for k in range(8):
            nc.sync.dma_start(xT[k][:, 0:512], xt_d.ap()[k * P : (k + 1) * P, 0:512])
        for k in range(8):
            nc.scalar.dma_start(wqk_t[k][:], wqk_d.ap()[k * P : (k + 1) * P, :])
            nc.scalar.dma_start(wv_t[k][:], wv_d.ap()[k * P : (k + 1) * P, :])
        for m in range(8):
            nc.gpsimd.dma_start(bq[m][:], bqk_d.ap()[m])
        for q in range(1, 4):
            for k in range(8):
                nc.sync.dma_start(
                    xT[k][:, q * 512 : (q + 1) * 512],
                    xt_d.ap()[k * P : (k + 1) * P, q * 512 : (q + 1) * 512],
                )
        for hp in range(4):
            nc.gpsimd.dma_start(wproj_t[hp][:], wproj_d.ap()[hp * P : (hp + 1) * P, :])
        nc.gpsimd.dma_start(beta_b[0:1, :], beta_d.ap())
        nc.gpsimd.partition_broadcast(beta_b[:], beta_b[0:1, :], channels=P)

        def vproj(q):
            # v for t-tiles of quarter q: vv[i] [keys=128, (mp h 65)]
            for il in range(4):
                i = q * 4 + il
                ps = spp.tile([P, 512], f32, tag="sp", name=f"vp{i}")
                for k in range(8):
                    nc.tensor.matmul(
                        ps[:],
                        xT[k][:, i * P : (i + 1) * P],
                        wv_t[k][:],
                        start=(k == 0), stop=(k == 7),
                    )
                src = ps[:].rearrange("p (mp h d) -> p mp h d", mp=4, h=2)
                dst = vv[i][:].rearrange("p (mp h d) -> p mp h d", mp=4, h=2)
                nc.vector.tensor_copy(dst[:, :, :, 0:64], src[:, :, :, :])

        def qkproj(n):
            ns = slice(n * 512, (n + 1) * 512)
            for m in range(8):
                ps = spp.tile([P, 512], f32, tag="sp", name=f"qkp{m}n{n}")
                for k in range(8):
                    nc.tensor.matmul(
                        ps[:],
                        wqk_t[k][:, m * P : (m + 1) * P],
                        xT[k][:, ns],
                        start=(k == 0), stop=(k == 7),
                    )
                nc.vector.tensor_scalar_add(qkT[m][:, ns], ps[:], bq[m][:])

        def attn(J):
            Js = slice(J * 512, (J + 1) * 512)
            nj = 4 * J + 4
            outN = [onp.tile([P, 512], bf16, tag=f"outN{mp}", name=f"outN{mp}J{J}")
                    for mp in range(4)]
            for mp in range(4):
                qs, ks = qkT[mp], qkT[4 + mp]
                ouA = oups.tile([65, 512], f32, tag="ouA")
                ouB = oups.tile([65, 512], f32, tag="ouB")
                for j in range(nj):
                    sT = stps.tile([P, 1024], f32, tag="sT")
                    js = slice(j * P, (j + 1) * P)
                    nc.tensor.matmul(
                        sT[:, 0:512], ks[0:64, js], qs[0:64, Js],
                        start=True, stop=True, tile_position=(0, 0),
                    )
                    nc.tensor.matmul(
                        sT[:, 512:1024], ks[64:128, js], qs[64:128, Js],
                        start=True, stop=True, tile_position=(64, 0),
                    )
                    at = atp.tile([P, 1024], bf16, tag="at")
                    i = j - 4 * J
                    if i > 0:
                        c0 = 128 * i
                        src_v = sT[:].rearrange("p (h c) -> p h c", h=2)
                        dst_v = at[:].rearrange("p (h c) -> p h c", h=2)
                        nc.scalar.activation(
                            dst_v[:, :, c0:512], src_v[:, :, c0:512],
                            EXP, bias=0.0, scale=0.125,
                        )
                    else:
                        nc.scalar.activation(at[:], sT[:], EXP, bias=0.0, scale=0.125)
                    if i >= 0:
                        for h0 in (0, 512):
                            c0 = h0 + 128 * i
                            if i > 0:
                                nc.vector.tensor_copy(
                                    at[:, h0 : h0 + 128 * i],
                                    zeros384[:, 0 : 128 * i],
                                )
                            nc.vector.tensor_mul(
                                at[:, c0 : c0 + 128],
                                at[:, c0 : c0 + 128], mask_tri[:],
                            )
                    if mp == 0 and J == 0 and j == 0:
                        _dbg(nc, "at000", at[:])
                    nc.tensor.matmul(
                        ouA[:], vv[j][:, 130 * mp : 130 * mp + 65],
                        at[:, 0:512],
                        start=(j == 0), stop=(j == nj - 1),
                    )
                    nc.tensor.matmul(
                        ouB[:], vv[j][:, 130 * mp + 65 : 130 * mp + 130],
                        at[:, 512:1024],
                        start=(j == 0), stop=(j == nj - 1),
                    )
                # normalization: denominator (psum row 64) reciprocal,
                # broadcast, scale. B half normalized at partitions 0:64 then
                # DMA-shifted to outN partitions 64:128.
                dA = nrm.tile([1, 512], f32, tag="dA")
                dB = nrm.tile([1, 512], f32, tag="dB")
                nc.any.tensor_copy(dA[:], ouA[64:65, :])
                nc.any.tensor_copy(dB[:], ouB[64:65, :])
                dRA = nrm.tile([1, 512], f32, tag="dRA")
                dRB = nrm.tile([1, 512], f32, tag="dRB")
                nc.vector.reciprocal_approx_fast(dRA[:], dA[:])
                nc.vector.reciprocal_approx_fast(dRB[:], dB[:])
                cA = nrm.tile([1, 512], bf16, tag="cA")
                cB = nrm.tile([1, 512], bf16, tag="cB")
                nc.vector.tensor_copy(cA[:], dRA[:])
                nc.vector.tensor_copy(cB[:], dRB[:])
                bcA = nrm.tile([64, 512], bf16, tag="bcA")
                bcB = nrm.tile([64, 512], bf16, tag="bcB")
                nc.gpsimd.partition_broadcast(bcA[:, :], cA[:], channels=64)
                nc.gpsimd.partition_broadcast(bcB[:, :], cB[:], channels=64)
                # normalize directly on psum eviction (one TT per head half)
                tbB = nrm.tile([64, 512], bf16, tag="tbB")
                nc.vector.tensor_mul(outN[mp][0:64, :], ouA[0:64, :], bcA[:, :])
                nc.vector.tensor_mul(tbB[:], ouB[0:64, :], bcB[:, :])
                nc.sync.dma_start(outN[mp][64:128, :], tbB[:])
            return outN

        def proj(J, outN):
            for i2 in range(4):
                for n in range(2):
                    ps = spp.tile([P, 512], f32, tag="sp", name=f"fp{J}i{i2}n{n}")
                    for hp in range(4):
                        nc.tensor.matmul(
                            ps[:],
                            outN[hp][:, i2 * P : (i2 + 1) * P],
                            wproj_t[hp][:, n * 512 : (n + 1) * 512],
                            start=(hp == 0), stop=(hp == 3),
                        )
                    fin = finp.tile([P, 512], bf16, tag="fin")
                    nc.vector.tensor_add(fin[:], ps[:], beta_b[:, n * 512 : (n + 1) * 512])
                    nc.sync.dma_start(
                        rs_in[J][i2 * P : (i2 + 1) * P, n * 512 : (n + 1) * 512],
                        fin[:],
                    )

        # ---------------- main pipeline ----------------
        vproj(0)
        qkproj(0)
        for J in range(4):
            outN = attn(J)
            # emit next round's projections before proj(J): they have no
            # dependency on outN, so they fill the PE while the mp=3
            # normalization chain completes.
            if J < 3:
                vproj(J + 1)
                qkproj(J + 1)
            proj(J, outN)
            if globals().get("_NO_COLLECTIVE"):
                nc.sync.dma_start(
                    out_d.ap()[J * 256 : (J + 1) * 256, :], rs_in[J][0:256, :]
                )
            else:
                nc.gpsimd.collective_compute(
                    "ReduceScatter", mybir.AluOpType.add,
                    replica_groups=[[0, 1], [2, 3], [4, 5], [6, 7]],
                    ins=[rs_in[J].opt()], outs=[rs_out[J].opt()],
                )
                nc.sync.dma_start(out_d.ap()[J * 256 : (J + 1) * 256, :], rs_out[J][:])
        _dbg(nc, "qkT0", qkT[0][:])
        _dbg(nc, "qkT4", qkT[4][:])
        _dbg(nc, "vv0", vv[0][:])
        _dbg(nc, "xT0", xT[0][:])


def _build():
    if "nc" in _CACHE:
        return _CACHE["nc"]
    nc = bacc.Bacc("TRN2", target_bir_lowering=False, debug=False, num_devices=NCORES)
    xt_d = nc.dram_tensor("x_t", [D, T], bf16, kind="ExternalInput")
    wqk_d = nc.dram_tensor("w_qk", [D, 1024], bf16, kind="ExternalInput")
    wv_d = nc.dram_tensor("w_v", [D, 512], bf16, kind="ExternalInput")
    bqk_d = nc.dram_tensor("b_qk", [8, P, 1], f32, kind="ExternalInput")
    wproj_d = nc.dram_tensor("w_proj", [512, D], bf16, kind="ExternalInput")
    beta_d = nc.dram_tensor("beta", [1, D], bf16, kind="ExternalInput")
    out_d = nc.dram_tensor("out", [T // 2, D], bf16, kind="ExternalOutput")
    with tile.TileContext(nc) as tc:
        _emit(nc, tc, xt_d, wqk_d, wv_d, bqk_d, wproj_d, beta_d, out_d)
    nc.compile()
    _CACHE["nc"] = nc
    return nc


def make_in_maps(x, w_qkv, b_qkv, w_proj, b_proj):
    x = np.asarray(x, np.float32)
    w_qkv = np.asarray(w_qkv, np.float32)
    b_qkv = np.asarray(b_qkv, np.float32)
    w_proj = np.asarray(w_proj, np.float32)
    b_proj = np.asarray(b_proj, np.float32)
    in_maps = []
    for c in range(NCORES):
        b, g = c // 2, c % 2
        qcols = slice(g * 512, (g + 1) * 512)
        kcols = slice(D + g * 512, D + (g + 1) * 512)
        vcols = slice(2 * D + g * 512, 2 * D + (g + 1) * 512)
        w_qk = np.concatenate([w_qkv[:, qcols], w_qkv[:, kcols]], axis=1)
        b_qk = np.concatenate([b_qkv[qcols], b_qkv[kcols]])
        wp = np.ascontiguousarray(w_proj[g * 512 : (g + 1) * 512, :])
        beta = wp.T @ b_qkv[vcols]
        if g == 0:
            beta = beta + b_proj
        in_maps.append({
            "x_t": np.ascontiguousarray(x[b].T).astype(ml_dtypes.bfloat16),
            "w_qk": np.ascontiguousarray(w_qk).astype(ml_dtypes.bfloat16),
            "w_v": np.ascontiguousarray(w_qkv[:, vcols]).astype(ml_dtypes.bfloat16),
            "b_qk": b_qk.reshape(8, P, 1),
            "w_proj": wp.astype(ml_dtypes.bfloat16),
            "beta": beta.reshape(1, D).astype(ml_dtypes.bfloat16),
        })
    return in_maps


def kernel(x, w_qkv, b_qkv, w_proj, b_proj, trace=False, **run_kwargs):
    global LAST_RESULTS
    nc = _build()
    in_maps = make_in_maps(x, w_qkv, b_qkv, w_proj, b_proj)
    res = run_bass_kernel_spmd(
        nc, in_maps, core_ids=list(range(NCORES)), trace=trace, **run_kwargs
    )
    LAST_RESULTS = res
    out = np.empty((B, T, D), np.float32)
    for b in range(B):
        ev = np.asarray(res.results[2 * b]["out"], dtype=np.float32)
        od = np.asarray(res.results[2 * b + 1]["out"], dtype=np.float32)
        for J in range(4):
            out[b, J * 512 : J * 512 + 256] = ev[J * 256 : (J + 1) * 256]
            out[b, J * 512 + 256 : (J + 1) * 512] = od[J * 256 : (J + 1) * 256]
    return out


# revision 12
# speedup vs baseline: 1.2067x; 1.2067x over previous
"""Causal self-attention Bass kernel for 8 trn2 NeuronCores.

Problem: B=4, T=2048, D=1024, H=16 causal self-attention (qkv proj + attn + out proj).

Sharding: core c = 2*b + g handles batch b (=c//2) and head-group g (=c%2, 8 heads).

Structure (J-pipelined, chunked collective):
  - x is transposed + bf16-cast on the HOST: xT [D, T] arrives ready for matmuls.
  - V projection per t-quarter: vv[i] [keys=128, 4mp x (65A|65B)] bf16, ones col
    at slot 64 of each 65-block (softmax denominator via the AV matmul).
  - QK projection per T-quarter n: qkT[m] [128 dims, T] bf16 (+bias).
  - Attention per query block J (512 q) x head pair mp: transposed-score flash
    loop over key blocks j<=diag; exp on ACT -> at bf16; causal masks on DVE;
    AV accumulates [65, 512] psum (row 64 = denominator).
  - Normalization per (mp, J): reciprocal_approx_fast on the denominator row,
    partition_broadcast, muls; B-half is normalized pre-shift then DMA'd to
    partitions 64:128.
  - Output projection per J -> rs_in[J] bf16 [512, 1024]; pairwise
    ReduceScatter per J (overlaps attention of J+1); even core gets rows
    [0:256) of each 512-row chunk, odd gets [256:512).
Host reassembles interleaved 256-row chunks per batch.
"""

from contextlib import ExitStack

import ml_dtypes
import numpy as np

import concourse.bass as bass
import concourse.mybir as mybir
import concourse.tile as tile
from concourse import bacc
from concourse.bass_utils import run_bass_kernel_spmd

B, T, D, H = 4, 2048, 1024, 16
HD = D // H  # 64
NCORES = 8
P = 128
f32 = mybir.dt.float32
f32r = mybir.dt.float32r
bf16 = mybir.dt.bfloat16
EXP = mybir.ActivationFunctionType.Exp

_CACHE = {}
LAST_RESULTS = None
_DEBUG_SINK = None


def _dbg(nc, name, ap):
    if _DEBUG_SINK is not None and name in _DEBUG_SINK:
        nc.sync.dma_start(_DEBUG_SINK[name].ap(), ap)


def _emit(nc, tc, xt_d, wqk_d, wv_d, bqk_d, wproj_d, beta_d, out_d):
    with ExitStack() as ctx:
        # ---------------- constants / persistent tiles ----------------
        const = ctx.enter_context(tc.tile_pool(name="const", bufs=1))
        mask_tri = const.tile([P, P], bf16, tag="mask_tri")
        nc.gpsimd.memset(mask_tri[:], 1.0)
        nc.gpsimd.affine_select(
            out=mask_tri[:], in_=mask_tri[:],
            compare_op=mybir.AluOpType.is_ge, fill=0.0,
            base=0, pattern=[[1, P]], channel_multiplier=-1,
        )
        zeros384 = const.tile([P, 384], bf16, tag="zeros384")
        nc.vector.memset(zeros384[:], 0.0)
        bq = [const.tile([P, 1], f32, tag=f"bq{m}", name=f"bq{m}") for m in range(8)]
        beta_b = const.tile([P, D], bf16, tag="beta_b")

        # persistent activations
        xt_pool = ctx.enter_context(tc.tile_pool(name="xt", bufs=1))
        xT = [xt_pool.tile([P, T], bf16, tag=f"xT{k}", name=f"xT{k}") for k in range(8)]
        qkt_pool = ctx.enter_context(tc.tile_pool(name="qkt", bufs=1))
        qkT = [qkt_pool.tile([P, T], bf16, tag=f"qkT{m}", name=f"qkT{m}") for m in range(8)]
        vv_pool = ctx.enter_context(tc.tile_pool(name="vv", bufs=1))
        vv = [vv_pool.tile([P, 520], bf16, tag=f"vv{i}", name=f"vv{i}") for i in range(16)]

        # weights
        wp = ctx.enter_context(tc.tile_pool(name="wts", bufs=1))
        wqk_t = [wp.tile([P, 1024], bf16, tag=f"wqk{k}", name=f"wqk{k}") for k in range(8)]
        wv_t = [wp.tile([P, 512], bf16, tag=f"wv{k}", name=f"wv{k}") for k in range(8)]
        wproj_t = [wp.tile([P, D], bf16, tag=f"wp{hp}", name=f"wp{hp}") for hp in range(4)]

        # ones columns of vv (denominator trick): col 64 of each 65-block
        ones8 = const.tile([P, 8], bf16, tag="ones8")
        nc.vector.memset(ones8[:], 1.0)
        ones_src = ones8[:].rearrange("p (mp h one) -> p mp h one", mp=4, h=2)
        for i in range(16):
            dst = vv[i][:].rearrange("p (mp h d) -> p mp h d", mp=4, h=2)
            nc.vector.tensor_copy(dst[:, :, :, 64:65], ones_src[:, :, :, :])

        # working pools
        onp = ctx.enter_context(tc.tile_pool(name="outn", bufs=3))
        atp = ctx.enter_context(tc.tile_pool(name="atp", bufs=4))
        nrm = ctx.enter_context(tc.tile_pool(name="nrm", bufs=2))
        finp = ctx.enter_context(tc.tile_pool(name="finp", bufs=3))

        spp = ctx.enter_context(tc.tile_pool(name="spp", bufs=2, space="PSUM"))
        stps = ctx.enter_context(tc.tile_pool(name="stps", bufs=2, space="PSUM"))
        oups = ctx.enter_context(tc.tile_pool(name="oups", bufs=1, space="PSUM"))

        dram = ctx.enter_context(tc.tile_pool(name="dram", bufs=1, space="DRAM"))
        # one RS chunk per 256-row half of each query block J (8 total):
        # separate tiles so each collective depends only on its own writes
        rs_in = [dram.tile([256, D], bf16, tag=f"rsi{c}", name=f"rsi{c}") for c in range(8)]
        rs_out = [dram.tile([128, D], bf16, tag=f"rso{c}", name=f"rso{c}") for c in range(8)]

        # ---------------- DMAs: quarter 0 + weights first ----------------
        for k in range(8):
            nc.sync.dma_start(xT[k][:, 0:512], xt_d.ap()[k * P : (k + 1) * P, 0:512])
        for k in range(8):
            nc.scalar.dma_start(wqk_t[k][:], wqk_d.ap()[k * P : (k + 1) * P, :])
            nc.scalar.dma_start(wv_t[k][:], wv_d.ap()[k * P : (k + 1) * P, :])
        for m in range(8):
            nc.gpsimd.dma_start(bq[m][:], bqk_d.ap()[m])
        for q in range(1, 4):
            for k in range(8):
                nc.sync.dma_start(
                    xT[k][:, q * 512 : (q + 1) * 512],
                    xt_d.ap()[k * P : (k + 1) * P, q * 512 : (q + 1) * 512],
                )
        for hp in range(4):
            nc.gpsimd.dma_start(wproj_t[hp][:], wproj_d.ap()[hp * P : (hp + 1) * P, :])
        nc.gpsimd.dma_start(beta_b[0:1, :], beta_d.ap())
        nc.gpsimd.partition_broadcast(beta_b[:], beta_b[0:1, :], channels=P)

        def vproj(q):
            # v for t-tiles of quarter q: vv[i] [keys=128, (mp h 65)]
            for il in range(4):
                i = q * 4 + il
                ps = spp.tile([P, 512], f32, tag="sp", name=f"vp{i}")
                for k in range(8):
                    nc.tensor.matmul(
                        ps[:],
                        xT[k][:, i * P : (i + 1) * P],
                        wv_t[k][:],
                        start=(k == 0), stop=(k == 7),
                    )
                src = ps[:].rearrange("p (mp h d) -> p mp h d", mp=4, h=2)
                dst = vv[i][:].rearrange("p (mp h d) -> p mp h d", mp=4, h=2)
                nc.vector.tensor_copy(dst[:, :, :, 0:64], src[:, :, :, :])

        def qkproj(n):
            ns = slice(n * 512, (n + 1) * 512)
            for m in range(8):
                ps = spp.tile([P, 512], f32, tag="sp", name=f"qkp{m}n{n}")
                for k in range(8):
                    nc.tensor.matmul(
                        ps[:],
                        wqk_t[k][:, m * P : (m + 1) * P],
                        xT[k][:, ns],
                        start=(k == 0), stop=(k == 7),
                    )
                nc.vector.tensor_scalar_add(qkT[m][:, ns], ps[:], bq[m][:])

        def attn(J):
            Js = slice(J * 512, (J + 1) * 512)
            nj = 4 * J + 4
            outN = [onp.tile([P, 512], bf16, tag=f"outN{mp}", name=f"outN{mp}J{J}")
                    for mp in range(4)]
            for mp in range(4):
                qs, ks = qkT[mp], qkT[4 + mp]
                ouA = oups.tile([65, 512], f32, tag="ouA")
                ouB = oups.tile([65, 512], f32, tag="ouB")
                for j in range(nj):
                    sT = stps.tile([P, 1024], f32, tag="sT")
                    js = slice(j * P, (j + 1) * P)
                    nc.tensor.matmul(
                        sT[:, 0:512], ks[0:64, js], qs[0:64, Js],
                        start=True, stop=True, tile_position=(0, 0),
                    )
                    nc.tensor.matmul(
                        sT[:, 512:1024], ks[64:128, js], qs[64:128, Js],
                        start=True, stop=True, tile_position=(64, 0),
                    )
                    at = atp.tile([P, 1024], bf16, tag="at")
                    i = j - 4 * J
                    if i > 0:
                        c0 = 128 * i
                        src_v = sT[:].rearrange("p (h c) -> p h c", h=2)
                        dst_v = at[:].rearrange("p (h c) -> p h c", h=2)
                        nc.scalar.activation(
                            dst_v[:, :, c0:512], src_v[:, :, c0:512],
                            EXP, bias=0.0, scale=0.125,
                        )
                    else:
                        nc.scalar.activation(at[:], sT[:], EXP, bias=0.0, scale=0.125)
                    if i >= 0:
                        for h0 in (0, 512):
                            c0 = h0 + 128 * i
                            if i > 0:
                                nc.vector.tensor_copy(
                                    at[:, h0 : h0 + 128 * i],
                                    zeros384[:, 0 : 128 * i],
                                )
                            nc.vector.tensor_mul(
                                at[:, c0 : c0 + 128],
                                at[:, c0 : c0 + 128], mask_tri[:],
                            )
                    if mp == 0 and J == 0 and j == 0:
                        _dbg(nc, "at000", at[:])
                    nc.tensor.matmul(
                        ouA[:], vv[j][:, 130 * mp : 130 * mp + 65],
                        at[:, 0:512],
                        start=(j == 0), stop=(j == nj - 1),
                    )
                    nc.tensor.matmul(
                        ouB[:], vv[j][:, 130 * mp + 65 : 130 * mp + 130],
                        at[:, 512:1024],
                        start=(j == 0), stop=(j == nj - 1),
                    )
                # normalization: denominator (psum row 64) reciprocal,
                # broadcast, scale. B half normalized at partitions 0:64 then
                # DMA-shifted to outN partitions 64:128.
                dA = nrm.tile([1, 512], f32, tag="dA")
                dB = nrm.tile([1, 512], f32, tag="dB")
                nc.any.tensor_copy(dA[:], ouA[64:65, :])
                nc.any.tensor_copy(dB[:], ouB[64:65, :])
                dRA = nrm.tile([1, 512], f32, tag="dRA")
                dRB = nrm.tile([1, 512], f32, tag="dRB")
                nc.vector.reciprocal_approx_fast(dRA[:], dA[:])
                nc.vector.reciprocal_approx_fast(dRB[:], dB[:])
                cA = nrm.tile([1, 512], bf16, tag="cA")
                cB = nrm.tile([1, 512], bf16, tag="cB")
                nc.vector.tensor_copy(cA[:], dRA[:])
                nc.vector.tensor_copy(cB[:], dRB[:])
                bcA = nrm.tile([64, 512], bf16, tag="bcA")
                bcB = nrm.tile([64, 512], bf16, tag="bcB")
                nc.gpsimd.partition_broadcast(bcA[:, :], cA[:], channels=64)
                nc.gpsimd.partition_broadcast(bcB[:, :], cB[:], channels=64)
                # evict psum promptly (frees the AV banks for the next head
                # pair), then normalize on SBUF
                tbA = nrm.tile([64, 512], bf16, tag="tbA")
                tbB = nrm.tile([64, 512], bf16, tag="tbB")
                nc.any.tensor_copy(tbA[:], ouA[0:64, :])
                nc.any.tensor_copy(tbB[:], ouB[0:64, :])
                nc.vector.tensor_mul(outN[mp][0:64, :], tbA[:], bcA[:, :])
                nc.vector.tensor_mul(tbB[:], tbB[:], bcB[:, :])
                nc.sync.dma_start(outN[mp][64:128, :], tbB[:])
            return outN

        def rs_chunk(c):
            if globals().get("_NO_COLLECTIVE"):
                nc.sync.dma_start(out_d.ap()[c * P : (c + 1) * P, :], rs_in[c][0:P, :])
            else:
                nc.gpsimd.collective_compute(
                    "ReduceScatter", mybir.AluOpType.add,
                    replica_groups=[[0, 1], [2, 3], [4, 5], [6, 7]],
                    ins=[rs_in[c].opt()], outs=[rs_out[c].opt()],
                )
                nc.sync.dma_start(out_d.ap()[c * P : (c + 1) * P, :], rs_out[c][:])

        def proj(J, outN):
            for i2 in range(4):
                c = 2 * J + i2 // 2  # 256-row RS chunk index
                for n in range(2):
                    ps = spp.tile([P, 512], f32, tag="sp", name=f"fp{J}i{i2}n{n}")
                    for hp in range(4):
                        nc.tensor.matmul(
                            ps[:],
                            outN[hp][:, i2 * P : (i2 + 1) * P],
                            wproj_t[hp][:, n * 512 : (n + 1) * 512],
                            start=(hp == 0), stop=(hp == 3),
                        )
                    fin = finp.tile([P, 512], bf16, tag="fin")
                    nc.vector.tensor_add(fin[:], ps[:], beta_b[:, n * 512 : (n + 1) * 512])
                    nc.sync.dma_start(
                        rs_in[c][(i2 % 2) * P : (i2 % 2 + 1) * P, n * 512 : (n + 1) * 512],
                        fin[:],
                    )
                if i2 % 2 == 1:
                    rs_chunk(c)

        # ---------------- main pipeline ----------------
        vproj(0)
        qkproj(0)
        for J in range(4):
            outN = attn(J)
            # emit next round's projections before proj(J): they have no
            # dependency on outN, so they fill the PE while the mp=3
            # normalization chain completes.
            if J < 3:
                vproj(J + 1)
                qkproj(J + 1)
            proj(J, outN)
        _dbg(nc, "qkT0", qkT[0][:])
        _dbg(nc, "qkT4", qkT[4][:])
        _dbg(nc, "vv0", vv[0][:])
        _dbg(nc, "xT0", xT[0][:])


def _build():
    if "nc" in _CACHE:
        return _CACHE["nc"]
    nc = bacc.Bacc("TRN2", target_bir_lowering=False, debug=False, num_devices=NCORES)
    xt_d = nc.dram_tensor("x_t", [D, T], bf16, kind="ExternalInput")
    wqk_d = nc.dram_tensor("w_qk", [D, 1024], bf16, kind="ExternalInput")
    wv_d = nc.dram_tensor("w_v", [D, 512], bf16, kind="ExternalInput")
    bqk_d = nc.dram_tensor("b_qk", [8, P, 1], f32, kind="ExternalInput")
    wproj_d = nc.dram_tensor("w_proj", [512, D], bf16, kind="ExternalInput")
    beta_d = nc.dram_tensor("beta", [1, D], bf16, kind="ExternalInput")
    out_d = nc.dram_tensor("out", [T // 2, D], bf16, kind="ExternalOutput")
    with tile.TileContext(nc) as tc:
        _emit(nc, tc, xt_d, wqk_d, wv_d, bqk_d, wproj_d, beta_d, out_d)
    nc.compile()
    _CACHE["nc"] = nc
    return nc


def make_in_maps(x, w_qkv, b_qkv, w_proj, b_proj):
    x = np.asarray(x, np.float32)
    w_qkv = np.asarray(w_qkv, np.float32)
    b_qkv = np.asarray(b_qkv, np.float32)
    w_proj = np.asarray(w_proj, np.float32)
    b_proj = np.asarray(b_proj, np.float32)
    in_maps = []
    for c in range(NCORES):
        b, g = c // 2, c % 2
        qcols = slice(g * 512, (g + 1) * 512)
        kcols = slice(D + g * 512, D + (g + 1) * 512)
        vcols = slice(2 * D + g * 512, 2 * D + (g + 1) * 512)
        w_qk = np.concatenate([w_qkv[:, qcols], w_qkv[:, kcols]], axis=1)
        b_qk = np.concatenate([b_qkv[qcols], b_qkv[kcols]])
        wp = np.ascontiguousarray(w_proj[g * 512 : (g + 1) * 512, :])
        beta = wp.T @ b_qkv[vcols]
        if g == 0:
            beta = beta + b_proj
        in_maps.append({
            "x_t": np.ascontiguousarray(x[b].T).astype(ml_dtypes.bfloat16),
            "w_qk": np.ascontiguousarray(w_qk).astype(ml_dtypes.bfloat16),
            "w_v": np.ascontiguousarray(w_qkv[:, vcols]).astype(ml_dtypes.bfloat16),
            "b_qk": b_qk.reshape(8, P, 1),
            "w_proj": wp.astype(ml_dtypes.bfloat16),
            "beta": beta.reshape(1, D).astype(ml_dtypes.bfloat16),
        })
    return in_maps


def kernel(x, w_qkv, b_qkv, w_proj, b_proj, trace=False, **run_kwargs):
    global LAST_RESULTS
    nc = _build()
    in_maps = make_in_maps(x, w_qkv, b_qkv, w_proj, b_proj)
    res = run_bass_kernel_spmd(
        nc, in_maps, core_ids=list(range(NCORES)), trace=trace, **run_kwargs
    )
    LAST_RESULTS = res
    out = np.empty((B, T, D), np.float32)
    for b in range(B):
        ev = np.asarray(res.results[2 * b]["out"], dtype=np.float32)
        od = np.asarray(res.results[2 * b + 1]["out"], dtype=np.float32)
        for c in range(8):
            out[b, c * 256 : c * 256 + 128] = ev[c * 128 : (c + 1) * 128]
            out[b, c * 256 + 128 : (c + 1) * 256] = od[c * 128 : (c + 1) * 128]
    return out


# revision 17
# speedup vs baseline: 1.2110x; 1.0036x over previous
"""Causal self-attention Bass kernel for 8 trn2 NeuronCores.

Problem: B=4, T=2048, D=1024, H=16 causal self-attention (qkv proj + attn + out proj).

Sharding: core c = 2*b + g handles batch b (=c//2) and head-group g (=c%2, 8 heads).

Structure (J-pipelined, chunked collective):
  - x is transposed + bf16-cast on the HOST: xT [D, T] arrives ready for matmuls.
  - V projection per t-quarter: vv[i] [keys=128, 4mp x (65A|65B)] bf16, ones col
    at slot 64 of each 65-block (softmax denominator via the AV matmul).
  - QK projection per T-quarter n: qkT[m] [128 dims, T] bf16 (+bias).
  - Attention per query block J (512 q) x head pair mp: transposed-score flash
    loop over key blocks j<=diag; exp on ACT -> at bf16; causal masks on DVE;
    AV accumulates [65, 512] psum (row 64 = denominator).
  - Normalization per (mp, J): reciprocal_approx_fast on the denominator row,
    partition_broadcast, muls; B-half is normalized pre-shift then DMA'd to
    partitions 64:128.
  - Output projection per J -> rs_in[J] bf16 [512, 1024]; pairwise
    ReduceScatter per J (overlaps attention of J+1); even core gets rows
    [0:256) of each 512-row chunk, odd gets [256:512).
Host reassembles interleaved 256-row chunks per batch.
"""

from contextlib import ExitStack

import ml_dtypes
import numpy as np

import concourse.bass as bass
import concourse.mybir as mybir
import concourse.tile as tile
from concourse import bacc
from concourse.bass_utils import run_bass_kernel_spmd

B, T, D, H = 4, 2048, 1024, 16
HD = D // H  # 64
NCORES = 8
P = 128
f32 = mybir.dt.float32
f32r = mybir.dt.float32r
bf16 = mybir.dt.bfloat16
EXP = mybir.ActivationFunctionType.Exp

_CACHE = {}
LAST_RESULTS = None
_DEBUG_SINK = None


def _dbg(nc, name, ap):
    if _DEBUG_SINK is not None and name in _DEBUG_SINK:
        nc.sync.dma_start(_DEBUG_SINK[name].ap(), ap)


def _emit(nc, tc, xt_d, wqk_d, wv_d, bqk_d, wproj_d, beta_d, out_d):
    with ExitStack() as ctx:
        # ---------------- constants / persistent tiles ----------------
        const = ctx.enter_context(tc.tile_pool(name="const", bufs=1))
        mask_tri = const.tile([P, P], bf16, tag="mask_tri")
        nc.gpsimd.memset(mask_tri[:], 1.0)
        nc.gpsimd.affine_select(
            out=mask_tri[:], in_=mask_tri[:],
            compare_op=mybir.AluOpType.is_ge, fill=0.0,
            base=0, pattern=[[1, P]], channel_multiplier=-1,
        )
        zeros384 = const.tile([P, 384], bf16, tag="zeros384")
        nc.vector.memset(zeros384[:], 0.0)
        bq = [const.tile([P, 1], f32, tag=f"bq{m}", name=f"bq{m}") for m in range(8)]
        beta_b = const.tile([P, D], bf16, tag="beta_b")

        # persistent activations
        xt_pool = ctx.enter_context(tc.tile_pool(name="xt", bufs=1))
        xT = [xt_pool.tile([P, T], bf16, tag=f"xT{k}", name=f"xT{k}") for k in range(8)]
        qkt_pool = ctx.enter_context(tc.tile_pool(name="qkt", bufs=1))
        qkT = [qkt_pool.tile([P, T], bf16, tag=f"qkT{m}", name=f"qkT{m}") for m in range(8)]
        vv_pool = ctx.enter_context(tc.tile_pool(name="vv", bufs=1))
        vv = [vv_pool.tile([P, 520], bf16, tag=f"vv{i}", name=f"vv{i}") for i in range(16)]

        # weights
        wp = ctx.enter_context(tc.tile_pool(name="wts", bufs=1))
        wqk_t = [wp.tile([P, 1024], bf16, tag=f"wqk{k}", name=f"wqk{k}") for k in range(8)]
        wv_t = [wp.tile([P, 512], bf16, tag=f"wv{k}", name=f"wv{k}") for k in range(8)]
        wproj_t = [wp.tile([P, D], bf16, tag=f"wp{hp}", name=f"wp{hp}") for hp in range(4)]

        # ones columns of vv (denominator trick): col 64 of each 65-block
        ones8 = const.tile([P, 8], bf16, tag="ones8")
        nc.vector.memset(ones8[:], 1.0)
        ones_src = ones8[:].rearrange("p (mp h one) -> p mp h one", mp=4, h=2)
        for i in range(16):
            dst = vv[i][:].rearrange("p (mp h d) -> p mp h d", mp=4, h=2)
            nc.vector.tensor_copy(dst[:, :, :, 64:65], ones_src[:, :, :, :])

        # working pools
        onp = ctx.enter_context(tc.tile_pool(name="outn", bufs=3))
        atp = ctx.enter_context(tc.tile_pool(name="atp", bufs=4))
        nrm = ctx.enter_context(tc.tile_pool(name="nrm", bufs=2))
        finp = ctx.enter_context(tc.tile_pool(name="finp", bufs=3))

        spp = ctx.enter_context(tc.tile_pool(name="spp", bufs=2, space="PSUM"))
        stps = ctx.enter_context(tc.tile_pool(name="stps", bufs=2, space="PSUM"))
        oups = ctx.enter_context(tc.tile_pool(name="oups", bufs=1, space="PSUM"))

        dram = ctx.enter_context(tc.tile_pool(name="dram", bufs=1, space="DRAM"))
        # one RS chunk per 256-row half of each query block J (8 total):
        # separate tiles so each collective depends only on its own writes
        rs_in = [dram.tile([256, D], bf16, tag=f"rsi{c}", name=f"rsi{c}") for c in range(8)]
        rs_out = [dram.tile([128, D], bf16, tag=f"rso{c}", name=f"rso{c}") for c in range(8)]

        # ---------------- DMAs: quarter 0 + weights first ----------------
        for k in range(8):
            nc.sync.dma_start(xT[k][:, 0:512], xt_d.ap()[k * P : (k + 1) * P, 0:512])
        for k in range(8):
            nc.scalar.dma_start(wqk_t[k][:], wqk_d.ap()[k * P : (k + 1) * P, :])
            nc.scalar.dma_start(wv_t[k][:], wv_d.ap()[k * P : (k + 1) * P, :])
        for m in range(8):
            nc.gpsimd.dma_start(bq[m][:], bqk_d.ap()[m])
        for q in range(1, 4):
            for k in range(8):
                nc.sync.dma_start(
                    xT[k][:, q * 512 : (q + 1) * 512],
                    xt_d.ap()[k * P : (k + 1) * P, q * 512 : (q + 1) * 512],
                )
        for hp in range(4):
            nc.gpsimd.dma_start(wproj_t[hp][:], wproj_d.ap()[hp * P : (hp + 1) * P, :])
        nc.gpsimd.dma_start(beta_b[0:1, :], beta_d.ap())
        nc.gpsimd.partition_broadcast(beta_b[:], beta_b[0:1, :], channels=P)

        def vproj(q):
            # v for t-tiles of quarter q: vv[i] [keys=128, (mp h 65)]
            for il in range(4):
                i = q * 4 + il
                ps = spp.tile([P, 512], f32, tag="sp", name=f"vp{i}")
                for k in range(8):
                    nc.tensor.matmul(
                        ps[:],
                        xT[k][:, i * P : (i + 1) * P],
                        wv_t[k][:],
                        start=(k == 0), stop=(k == 7),
                    )
                src = ps[:].rearrange("p (mp h d) -> p mp h d", mp=4, h=2)
                dst = vv[i][:].rearrange("p (mp h d) -> p mp h d", mp=4, h=2)
                nc.vector.tensor_copy(dst[:, :, :, 0:64], src[:, :, :, :])

        def qkproj(n, ms=range(8)):
            ns = slice(n * 512, (n + 1) * 512)
            for m in ms:
                ps = spp.tile([P, 512], f32, tag="sp", name=f"qkp{m}n{n}")
                for k in range(8):
                    nc.tensor.matmul(
                        ps[:],
                        wqk_t[k][:, m * P : (m + 1) * P],
                        xT[k][:, ns],
                        start=(k == 0), stop=(k == 7),
                    )
                nc.vector.tensor_scalar_add(qkT[m][:, ns], ps[:], bq[m][:])

        def attn(J, fillers=()):
            Js = slice(J * 512, (J + 1) * 512)
            nj = 4 * J + 4
            fillers = list(fillers)
            outN = [onp.tile([P, 512], bf16, tag=f"outN{mp}", name=f"outN{mp}J{J}")
                    for mp in range(4)]
            for mp in range(4):
                qs, ks = qkT[mp], qkT[4 + mp]
                ouA = oups.tile([65, 512], f32, tag="ouA")
                ouB = oups.tile([65, 512], f32, tag="ouB")
                for j in range(nj):
                    sT = stps.tile([P, 1024], f32, tag="sT")
                    js = slice(j * P, (j + 1) * P)
                    nc.tensor.matmul(
                        sT[:, 0:512], ks[0:64, js], qs[0:64, Js],
                        start=True, stop=True, tile_position=(0, 0),
                    )
                    nc.tensor.matmul(
                        sT[:, 512:1024], ks[64:128, js], qs[64:128, Js],
                        start=True, stop=True, tile_position=(64, 0),
                    )
                    at = atp.tile([P, 1024], bf16, tag="at")
                    i = j - 4 * J
                    if i > 0:
                        c0 = 128 * i
                        src_v = sT[:].rearrange("p (h c) -> p h c", h=2)
                        dst_v = at[:].rearrange("p (h c) -> p h c", h=2)
                        nc.scalar.activation(
                            dst_v[:, :, c0:512], src_v[:, :, c0:512],
                            EXP, bias=0.0, scale=0.125,
                        )
                    else:
                        nc.scalar.activation(at[:], sT[:], EXP, bias=0.0, scale=0.125)
                    if i >= 0:
                        for h0 in (0, 512):
                            c0 = h0 + 128 * i
                            if i > 0:
                                nc.vector.tensor_copy(
                                    at[:, h0 : h0 + 128 * i],
                                    zeros384[:, 0 : 128 * i],
                                )
                            nc.vector.tensor_mul(
                                at[:, c0 : c0 + 128],
                                at[:, c0 : c0 + 128], mask_tri[:],
                            )
                    if mp == 0 and J == 0 and j == 0:
                        _dbg(nc, "at000", at[:])
                    nc.tensor.matmul(
                        ouA[:], vv[j][:, 130 * mp : 130 * mp + 65],
                        at[:, 0:512],
                        start=(j == 0), stop=(j == nj - 1),
                    )
                    nc.tensor.matmul(
                        ouB[:], vv[j][:, 130 * mp + 65 : 130 * mp + 130],
                        at[:, 512:1024],
                        start=(j == 0), stop=(j == nj - 1),
                    )
                # normalization: denominator (psum row 64) reciprocal,
                # broadcast, scale. B half normalized at partitions 0:64 then
                # DMA-shifted to outN partitions 64:128.
                dA = nrm.tile([1, 512], f32, tag="dA")
                dB = nrm.tile([1, 512], f32, tag="dB")
                nc.any.tensor_copy(dA[:], ouA[64:65, :])
                nc.any.tensor_copy(dB[:], ouB[64:65, :])
                dRA = nrm.tile([1, 512], f32, tag="dRA")
                dRB = nrm.tile([1, 512], f32, tag="dRB")
                nc.vector.reciprocal_approx_fast(dRA[:], dA[:])
                nc.vector.reciprocal_approx_fast(dRB[:], dB[:])
                cA = nrm.tile([1, 512], bf16, tag="cA")
                cB = nrm.tile([1, 512], bf16, tag="cB")
                nc.vector.tensor_copy(cA[:], dRA[:])
                nc.vector.tensor_copy(cB[:], dRB[:])
                bcA = nrm.tile([64, 512], bf16, tag="bcA")
                bcB = nrm.tile([64, 512], bf16, tag="bcB")
                nc.gpsimd.partition_broadcast(bcA[:, :], cA[:], channels=64)
                nc.gpsimd.partition_broadcast(bcB[:, :], cB[:], channels=64)
                # evict psum promptly (frees the AV banks for the next head
                # pair), then normalize on SBUF
                tbA = nrm.tile([64, 512], bf16, tag="tbA")
                tbB = nrm.tile([64, 512], bf16, tag="tbB")
                nc.any.tensor_copy(tbA[:], ouA[0:64, :])
                nc.any.tensor_copy(tbB[:], ouB[0:64, :])
                nc.vector.tensor_mul(outN[mp][0:64, :], tbA[:], bcA[:, :])
                nc.vector.tensor_mul(tbB[:], tbB[:], bcB[:, :])
                nc.sync.dma_start(outN[mp][64:128, :], tbB[:])
                # interleave independent fill work (prev proj / next
                # projections) so the in-order PE queue never starves on the
                # normalization chain or exp latency
                if fillers:
                    fillers.pop(0)()
            while fillers:
                fillers.pop(0)()
            return outN

        def rs_chunk(c):
            if globals().get("_NO_COLLECTIVE"):
                nc.sync.dma_start(out_d.ap()[c * P : (c + 1) * P, :], rs_in[c][0:P, :])
            else:
                nc.gpsimd.collective_compute(
                    "ReduceScatter", mybir.AluOpType.add,
                    replica_groups=[[0, 1], [2, 3], [4, 5], [6, 7]],
                    ins=[rs_in[c].opt()], outs=[rs_out[c].opt()],
                )
                nc.sync.dma_start(out_d.ap()[c * P : (c + 1) * P, :], rs_out[c][:])

        def proj(J, outN):
            for i2 in range(4):
                c = 2 * J + i2 // 2  # 256-row RS chunk index
                for n in range(2):
                    ps = spp.tile([P, 512], f32, tag="sp", name=f"fp{J}i{i2}n{n}")
                    for hp in range(4):
                        nc.tensor.matmul(
                            ps[:],
                            outN[hp][:, i2 * P : (i2 + 1) * P],
                            wproj_t[hp][:, n * 512 : (n + 1) * 512],
                            start=(hp == 0), stop=(hp == 3),
                        )
                    fin = finp.tile([P, 512], bf16, tag="fin")
                    nc.vector.tensor_add(fin[:], ps[:], beta_b[:, n * 512 : (n + 1) * 512])
                    nc.sync.dma_start(
                        rs_in[c][(i2 % 2) * P : (i2 % 2 + 1) * P, n * 512 : (n + 1) * 512],
                        fin[:],
                    )
                if i2 % 2 == 1:
                    rs_chunk(c)

        # ---------------- main pipeline ----------------
        vproj(0)
        qkproj(0)
        pending = None
        for J in range(4):
            fillers = []
            if pending is not None:
                Jp, outNp = pending
                fillers.append(lambda Jp=Jp, o=outNp: proj(Jp, o))
            if J < 3:
                fillers.append(lambda q=J + 1: vproj(q))
                fillers.append(lambda q=J + 1: qkproj(q, range(4)))
                fillers.append(lambda q=J + 1: qkproj(q, range(4, 8)))
            outN = attn(J, fillers)
            pending = (J, outN)
        proj(pending[0], pending[1])
        _dbg(nc, "qkT0", qkT[0][:])
        _dbg(nc, "qkT4", qkT[4][:])
        _dbg(nc, "vv0", vv[0][:])
        _dbg(nc, "xT0", xT[0][:])


def _build():
    if "nc" in _CACHE:
        return _CACHE["nc"]
    nc = bacc.Bacc("TRN2", target_bir_lowering=False, debug=False, num_devices=NCORES)
    xt_d = nc.dram_tensor("x_t", [D, T], bf16, kind="ExternalInput")
    wqk_d = nc.dram_tensor("w_qk", [D, 1024], bf16, kind="ExternalInput")
    wv_d = nc.dram_tensor("w_v", [D, 512], bf16, kind="ExternalInput")
    bqk_d = nc.dram_tensor("b_qk", [8, P, 1], f32, kind="ExternalInput")
    wproj_d = nc.dram_tensor("w_proj", [512, D], bf16, kind="ExternalInput")
    beta_d = nc.dram_tensor("beta", [1, D], bf16, kind="ExternalInput")
    out_d = nc.dram_tensor("out", [T // 2, D], bf16, kind="ExternalOutput")
    with tile.TileContext(nc) as tc:
        _emit(nc, tc, xt_d, wqk_d, wv_d, bqk_d, wproj_d, beta_d, out_d)
    nc.compile()
    _CACHE["nc"] = nc
    return nc


def make_in_maps(x, w_qkv, b_qkv, w_proj, b_proj):
    x = np.asarray(x, np.float32)
    w_qkv = np.asarray(w_qkv, np.float32)
    b_qkv = np.asarray(b_qkv, np.float32)
    w_proj = np.asarray(w_proj, np.float32)
    b_proj = np.asarray(b_proj, np.float32)
    in_maps = []
    for c in range(NCORES):
        b, g = c // 2, c % 2
        qcols = slice(g * 512, (g + 1) * 512)
        kcols = slice(D + g * 512, D + (g + 1) * 512)
        vcols = slice(2 * D + g * 512, 2 * D + (g + 1) * 512)
        w_qk = np.concatenate([w_qkv[:, qcols], w_qkv[:, kcols]], axis=1)
        b_qk = np.concatenate([b_qkv[qcols], b_qkv[kcols]])
        wp = np.ascontiguousarray(w_proj[g * 512 : (g + 1) * 512, :])
        beta = wp.T @ b_qkv[vcols]
        if g == 0:
            beta = beta + b_proj
        in_maps.append({
            "x_t": np.ascontiguousarray(x[b].T).astype(ml_dtypes.bfloat16),
            "w_qk": np.ascontiguousarray(w_qk).astype(ml_dtypes.bfloat16),
            "w_v": np.ascontiguousarray(w_qkv[:, vcols]).astype(ml_dtypes.bfloat16),
            "b_qk": b_qk.reshape(8, P, 1),
            "w_proj": wp.astype(ml_dtypes.bfloat16),
            "beta": beta.reshape(1, D).astype(ml_dtypes.bfloat16),
        })
    return in_maps


def kernel(x, w_qkv, b_qkv, w_proj, b_proj, trace=False, **run_kwargs):
    global LAST_RESULTS
    nc = _build()
    in_maps = make_in_maps(x, w_qkv, b_qkv, w_proj, b_proj)
    res = run_bass_kernel_spmd(
        nc, in_maps, core_ids=list(range(NCORES)), trace=trace, **run_kwargs
    )
    LAST_RESULTS = res
    out = np.empty((B, T, D), np.float32)
    for b in range(B):
        ev = np.asarray(res.results[2 * b]["out"], dtype=np.float32)
        od = np.asarray(res.results[2 * b + 1]["out"], dtype=np.float32)
        for c in range(8):
            out[b, c * 256 : c * 256 + 128] = ev[c * 128 : (c + 1) * 128]
            out[b, c * 256 + 128 : (c + 1) * 256] = od[c * 128 : (c + 1) * 128]
    return out


# revision 25
# speedup vs baseline: 1.2567x; 1.0377x over previous
"""Causal self-attention Bass kernel for 8 trn2 NeuronCores.

Problem: B=4, T=2048, D=1024, H=16 causal self-attention (qkv proj + attn + out proj).

Sharding: core c = 2*b + g handles batch b (=c//2) and head-group g (=c%2, 8 heads).

Structure (J-pipelined, chunked collective):
  - x is transposed + bf16-cast on the HOST: xT [D, T] arrives ready for matmuls.
  - V projection per t-quarter: vv[i] [keys=128, 4mp x (65A|65B)] bf16, ones col
    at slot 64 of each 65-block (softmax denominator via the AV matmul).
  - QK projection per T-quarter n: qkT[m] [128 dims, T] bf16 (+bias).
  - Attention per query block J (512 q) x head pair mp: transposed-score flash
    loop over key blocks j<=diag; exp on ACT -> at bf16; causal masks on DVE;
    AV accumulates [65, 512] psum (row 64 = denominator).
  - Normalization per (mp, J): reciprocal_approx_fast on the denominator row,
    partition_broadcast, muls; B-half is normalized pre-shift then DMA'd to
    partitions 64:128.
  - Output projection per J -> rs_in[J] bf16 [512, 1024]; pairwise
    ReduceScatter per J (overlaps attention of J+1); even core gets rows
    [0:256) of each 512-row chunk, odd gets [256:512).
Host reassembles interleaved 256-row chunks per batch.
"""

from contextlib import ExitStack

import ml_dtypes
import numpy as np

import concourse.bass as bass
import concourse.mybir as mybir
import concourse.tile as tile
from concourse import bacc
from concourse.bass_utils import run_bass_kernel_spmd

B, T, D, H = 4, 2048, 1024, 16
HD = D // H  # 64
NCORES = 8
P = 128
f32 = mybir.dt.float32
f32r = mybir.dt.float32r
bf16 = mybir.dt.bfloat16
EXP = mybir.ActivationFunctionType.Exp

_CACHE = {}
LAST_RESULTS = None
_DEBUG_SINK = None


def _dbg(nc, name, ap):
    if _DEBUG_SINK is not None and name in _DEBUG_SINK:
        nc.sync.dma_start(_DEBUG_SINK[name].ap(), ap)


def _emit(nc, tc, xt_d, wqk_d, wv_d, bqk_d, wproj_d, beta_d, out_d):
    with ExitStack() as ctx:
        # ---------------- constants / persistent tiles ----------------
        const = ctx.enter_context(tc.tile_pool(name="const", bufs=1))
        mask_tri = const.tile([P, P], bf16, tag="mask_tri")
        nc.gpsimd.memset(mask_tri[:], 1.0)
        nc.gpsimd.affine_select(
            out=mask_tri[:], in_=mask_tri[:],
            compare_op=mybir.AluOpType.is_ge, fill=0.0,
            base=0, pattern=[[1, P]], channel_multiplier=-1,
        )
        zeros384 = const.tile([P, 384], bf16, tag="zeros384")
        nc.vector.memset(zeros384[:], 0.0)
        bq = [const.tile([P, 1], f32, tag=f"bq{m}", name=f"bq{m}") for m in range(8)]
        beta_b = const.tile([P, D], bf16, tag="beta_b")

        # persistent activations
        xt_pool = ctx.enter_context(tc.tile_pool(name="xt", bufs=1))
        xT = [xt_pool.tile([P, T], bf16, tag=f"xT{k}", name=f"xT{k}") for k in range(8)]
        qkt_pool = ctx.enter_context(tc.tile_pool(name="qkt", bufs=1))
        qkT = [qkt_pool.tile([P, T], bf16, tag=f"qkT{m}", name=f"qkT{m}") for m in range(8)]
        vv_pool = ctx.enter_context(tc.tile_pool(name="vv", bufs=1))
        vv = [vv_pool.tile([P, 520], bf16, tag=f"vv{i}", name=f"vv{i}") for i in range(16)]

        # weights
        wp = ctx.enter_context(tc.tile_pool(name="wts", bufs=1))
        wqk_t = [wp.tile([P, 1024], bf16, tag=f"wqk{k}", name=f"wqk{k}") for k in range(8)]
        wv_t = [wp.tile([P, 512], bf16, tag=f"wv{k}", name=f"wv{k}") for k in range(8)]
        wproj_t = [wp.tile([P, D], bf16, tag=f"wp{hp}", name=f"wp{hp}") for hp in range(4)]

        # ones columns of vv (denominator trick): col 64 of each 65-block
        ones8 = const.tile([P, 8], bf16, tag="ones8")
        nc.vector.memset(ones8[:], 1.0)
        ones_src = ones8[:].rearrange("p (mp h one) -> p mp h one", mp=4, h=2)
        for i in range(16):
            dst = vv[i][:].rearrange("p (mp h d) -> p mp h d", mp=4, h=2)
            nc.vector.tensor_copy(dst[:, :, :, 64:65], ones_src[:, :, :, :])

        # working pools
        onp = ctx.enter_context(tc.tile_pool(name="outn", bufs=3))
        atp = ctx.enter_context(tc.tile_pool(name="atp", bufs=4))
        nrm = ctx.enter_context(tc.tile_pool(name="nrm", bufs=2))
        finp = ctx.enter_context(tc.tile_pool(name="finp", bufs=3))

        spp = ctx.enter_context(tc.tile_pool(name="spp", bufs=2, space="PSUM"))
        stps = ctx.enter_context(tc.tile_pool(name="stps", bufs=2, space="PSUM"))
        oups = ctx.enter_context(tc.tile_pool(name="oups", bufs=1, space="PSUM"))

        dram = ctx.enter_context(tc.tile_pool(name="dram", bufs=1, space="DRAM"))
        # one RS chunk per 256-row half of each query block J (8 total):
        # separate tiles so each collective depends only on its own writes
        rs_in = [dram.tile([256, D], bf16, tag=f"rsi{c}", name=f"rsi{c}") for c in range(8)]
        rs_out = [dram.tile([128, D], bf16, tag=f"rso{c}", name=f"rso{c}") for c in range(8)]

        # ~4us of dummy matmuls at the head of the PE queue: flips the HAM
        # clock gate to 8/8 while the first input DMAs are still in flight,
        # so the real matmuls start at 2.4 GHz.
        wps = spp.tile([P, 384], f32, tag="sp", name="warm")
        for d in range(12):
            nc.tensor.matmul(
                wps[:], zeros384[:, 0:128], zeros384[:],
                start=(d == 0), stop=(d == 11),
            )

        # ---------------- DMAs: quarter 0 + weights first ----------------
        for k in range(8):
            nc.sync.dma_start(xT[k][:, 0:512], xt_d.ap()[k * P : (k + 1) * P, 0:512])
        for k in range(8):
            nc.scalar.dma_start(wv_t[k][:], wv_d.ap()[k * P : (k + 1) * P, :])
        for k in range(8):
            nc.scalar.dma_start(wqk_t[k][:], wqk_d.ap()[k * P : (k + 1) * P, :])
        for m in range(8):
            nc.gpsimd.dma_start(bq[m][:], bqk_d.ap()[m])
        for q in range(1, 4):
            for k in range(8):
                nc.sync.dma_start(
                    xT[k][:, q * 512 : (q + 1) * 512],
                    xt_d.ap()[k * P : (k + 1) * P, q * 512 : (q + 1) * 512],
                )
        for hp in range(4):
            nc.gpsimd.dma_start(wproj_t[hp][:], wproj_d.ap()[hp * P : (hp + 1) * P, :])
        nc.gpsimd.dma_start(beta_b[0:1, :], beta_d.ap())
        nc.gpsimd.partition_broadcast(beta_b[:], beta_b[0:1, :], channels=P)

        def vproj(q, ils=range(4)):
            # v for t-tiles of quarter q: vv[i] [keys=128, (mp h 65)]
            for il in ils:
                i = q * 4 + il
                ps = spp.tile([P, 512], f32, tag="sp", name=f"vp{i}")
                for k in range(8):
                    nc.tensor.matmul(
                        ps[:],
                        xT[k][:, i * P : (i + 1) * P],
                        wv_t[k][:],
                        start=(k == 0), stop=(k == 7),
                    )
                src = ps[:].rearrange("p (mp h d) -> p mp h d", mp=4, h=2)
                dst = vv[i][:].rearrange("p (mp h d) -> p mp h d", mp=4, h=2)
                nc.vector.tensor_copy(dst[:, :, :, 0:64], src[:, :, :, :])

        def qkproj(n, ms=range(8)):
            ns = slice(n * 512, (n + 1) * 512)
            for m in ms:
                ps = spp.tile([P, 512], f32, tag="sp", name=f"qkp{m}n{n}")
                for k in range(8):
                    nc.tensor.matmul(
                        ps[:],
                        wqk_t[k][:, m * P : (m + 1) * P],
                        xT[k][:, ns],
                        start=(k == 0), stop=(k == 7),
                    )
                nc.vector.tensor_scalar_add(qkT[m][:, ns], ps[:], bq[m][:])

        def attn(J, fillers=()):
            Js = slice(J * 512, (J + 1) * 512)
            nj = 4 * J + 4
            fillers = list(fillers)
            outN = [onp.tile([P, 512], bf16, tag=f"outN{mp}", name=f"outN{mp}J{J}")
                    for mp in range(4)]
            for mp in range(4):
                qs, ks = qkT[mp], qkT[4 + mp]
                ouA = oups.tile([65, 512], f32, tag="ouA")
                ouB = oups.tile([65, 512], f32, tag="ouB")
                for j in range(nj):
                    sT = stps.tile([P, 1024], f32, tag="sT")
                    js = slice(j * P, (j + 1) * P)
                    nc.tensor.matmul(
                        sT[:, 0:512], ks[0:64, js], qs[0:64, Js],
                        start=True, stop=True, tile_position=(0, 0),
                    )
                    nc.tensor.matmul(
                        sT[:, 512:1024], ks[64:128, js], qs[64:128, Js],
                        start=True, stop=True, tile_position=(64, 0),
                    )
                    at = atp.tile([P, 1024], bf16, tag="at")
                    i = j - 4 * J
                    if i > 0:
                        c0 = 128 * i
                        src_v = sT[:].rearrange("p (h c) -> p h c", h=2)
                        dst_v = at[:].rearrange("p (h c) -> p h c", h=2)
                        nc.scalar.activation(
                            dst_v[:, :, c0:512], src_v[:, :, c0:512],
                            EXP, bias=0.0, scale=0.125,
                        )
                    else:
                        nc.scalar.activation(at[:], sT[:], EXP, bias=0.0, scale=0.125)
                    if i >= 0:
                        for h0 in (0, 512):
                            c0 = h0 + 128 * i
                            if i > 0:
                                nc.vector.tensor_copy(
                                    at[:, h0 : h0 + 128 * i],
                                    zeros384[:, 0 : 128 * i],
                                )
                            nc.vector.tensor_mul(
                                at[:, c0 : c0 + 128],
                                at[:, c0 : c0 + 128], mask_tri[:],
                            )
                    if mp == 0 and J == 0 and j == 0:
                        _dbg(nc, "at000", at[:])
                    nc.tensor.matmul(
                        ouA[:], vv[j][:, 130 * mp : 130 * mp + 65],
                        at[:, 0:512],
                        start=(j == 0), stop=(j == nj - 1),
                    )
                    nc.tensor.matmul(
                        ouB[:], vv[j][:, 130 * mp + 65 : 130 * mp + 130],
                        at[:, 512:1024],
                        start=(j == 0), stop=(j == nj - 1),
                    )
                    if J == 0 and fillers:
                        fillers.pop(0)()
                # normalization: denominator (psum row 64) reciprocal,
                # broadcast, scale. B half normalized at partitions 0:64 then
                # DMA-shifted to outN partitions 64:128.
                dA = nrm.tile([1, 512], f32, tag="dA")
                dB = nrm.tile([1, 512], f32, tag="dB")
                nc.any.tensor_copy(dA[:], ouA[64:65, :])
                nc.any.tensor_copy(dB[:], ouB[64:65, :])
                dRA = nrm.tile([1, 512], f32, tag="dRA")
                dRB = nrm.tile([1, 512], f32, tag="dRB")
                nc.vector.reciprocal_approx_fast(dRA[:], dA[:])
                nc.vector.reciprocal_approx_fast(dRB[:], dB[:])
                cA = nrm.tile([1, 512], bf16, tag="cA")
                cB = nrm.tile([1, 512], bf16, tag="cB")
                nc.vector.tensor_copy(cA[:], dRA[:])
                nc.vector.tensor_copy(cB[:], dRB[:])
                bcA = nrm.tile([64, 512], bf16, tag="bcA")
                bcB = nrm.tile([P, 512], bf16, tag="bcB")
                nc.gpsimd.partition_broadcast(bcA[:, :], cA[:], channels=64)
                nc.gpsimd.partition_broadcast(bcB[:, :], cB[:], channels=P)
                # evict psum promptly (frees the AV banks for the next head
                # pair); B half is DMA'd raw to partitions 64:128 early and
                # normalized in place (keeps the shift DMA off the chain tail)
                tbA = nrm.tile([64, 512], bf16, tag="tbA")
                tbB = nrm.tile([64, 512], bf16, tag="tbB")
                nc.any.tensor_copy(tbA[:], ouA[0:64, :])
                nc.any.tensor_copy(tbB[:], ouB[0:64, :])
                nc.sync.dma_start(outN[mp][64:128, :], tbB[:])
                nc.vector.tensor_mul(outN[mp][0:64, :], tbA[:], bcA[:, :])
                nc.vector.tensor_mul(
                    outN[mp][64:128, :], outN[mp][64:128, :], bcB[64:128, :]
                )
                # interleave independent fill work (prev proj / next
                # projections) so the in-order PE queue never starves on the
                # normalization chain or exp latency
                if fillers:
                    fillers.pop(0)()
            while fillers:
                fillers.pop(0)()
            return outN

        def rs_chunk(c):
            if globals().get("_NO_COLLECTIVE"):
                nc.sync.dma_start(out_d.ap()[c * P : (c + 1) * P, :], rs_in[c][0:P, :])
            else:
                nc.gpsimd.collective_compute(
                    "ReduceScatter", mybir.AluOpType.add,
                    replica_groups=[[0, 1], [2, 3], [4, 5], [6, 7]],
                    ins=[rs_in[c].opt()], outs=[rs_out[c].opt()],
                )
                nc.sync.dma_start(out_d.ap()[c * P : (c + 1) * P, :], rs_out[c][:])

        def proj(J, outN):
            for i2 in range(4):
                c = 2 * J + i2 // 2  # 256-row RS chunk index
                for n in range(2):
                    ps = spp.tile([P, 512], f32, tag="sp", name=f"fp{J}i{i2}n{n}")
                    for hp in range(4):
                        nc.tensor.matmul(
                            ps[:],
                            outN[hp][:, i2 * P : (i2 + 1) * P],
                            wproj_t[hp][:, n * 512 : (n + 1) * 512],
                            start=(hp == 0), stop=(hp == 3),
                        )
                    fin = finp.tile([P, 512], bf16, tag="fin")
                    nc.vector.tensor_add(fin[:], ps[:], beta_b[:, n * 512 : (n + 1) * 512])
                    nc.sync.dma_start(
                        rs_in[c][(i2 % 2) * P : (i2 % 2 + 1) * P, n * 512 : (n + 1) * 512],
                        fin[:],
                    )
                if i2 % 2 == 1:
                    rs_chunk(c)

        # ---------------- main pipeline ----------------
        vproj(0)
        qkproj(0)
        pending = None
        for J in range(4):
            fillers = []
            if pending is not None:
                Jp, outNp = pending
                fillers.append(lambda Jp=Jp, o=outNp: proj(Jp, o))
            if J == 0:
                # fine-grained fillers: consumed per key-block in attn(0),
                # which is short and exp-latency-bound
                for il in range(4):
                    fillers.append(lambda il=il: vproj(1, [il]))
                for m in range(8):
                    fillers.append(lambda m=m: qkproj(1, [m]))
            elif J < 3:
                fillers.append(lambda q=J + 1: vproj(q))
                fillers.append(lambda q=J + 1: qkproj(q, range(4)))
                fillers.append(lambda q=J + 1: qkproj(q, range(4, 8)))
            outN = attn(J, fillers)
            pending = (J, outN)
        proj(pending[0], pending[1])
        _dbg(nc, "qkT0", qkT[0][:])
        _dbg(nc, "qkT4", qkT[4][:])
        _dbg(nc, "vv0", vv[0][:])
        _dbg(nc, "xT0", xT[0][:])


def _build():
    if "nc" in _CACHE:
        return _CACHE["nc"]
    nc = bacc.Bacc("TRN2", target_bir_lowering=False, debug=False, num_devices=NCORES)
    xt_d = nc.dram_tensor("x_t", [D, T], bf16, kind="ExternalInput")
    wqk_d = nc.dram_tensor("w_qk", [D, 1024], bf16, kind="ExternalInput")
    wv_d = nc.dram_tensor("w_v", [D, 512], bf16, kind="ExternalInput")
    bqk_d = nc.dram_tensor("b_qk", [8, P, 1], f32, kind="ExternalInput")
    wproj_d = nc.dram_tensor("w_proj", [512, D], bf16, kind="ExternalInput")
    beta_d = nc.dram_tensor("beta", [1, D], bf16, kind="ExternalInput")
    out_d = nc.dram_tensor("out", [T // 2, D], bf16, kind="ExternalOutput")
    with tile.TileContext(nc) as tc:
        _emit(nc, tc, xt_d, wqk_d, wv_d, bqk_d, wproj_d, beta_d, out_d)
    nc.compile()
    _CACHE["nc"] = nc
    return nc


def make_in_maps(x, w_qkv, b_qkv, w_proj, b_proj):
    x = np.asarray(x, np.float32)
    w_qkv = np.asarray(w_qkv, np.float32)
    b_qkv = np.asarray(b_qkv, np.float32)
    w_proj = np.asarray(w_proj, np.float32)
    b_proj = np.asarray(b_proj, np.float32)
    in_maps = []
    for c in range(NCORES):
        b, g = c // 2, c % 2
        qcols = slice(g * 512, (g + 1) * 512)
        kcols = slice(D + g * 512, D + (g + 1) * 512)
        vcols = slice(2 * D + g * 512, 2 * D + (g + 1) * 512)
        w_qk = np.concatenate([w_qkv[:, qcols], w_qkv[:, kcols]], axis=1)
        b_qk = np.concatenate([b_qkv[qcols], b_qkv[kcols]])
        wp = np.ascontiguousarray(w_proj[g * 512 : (g + 1) * 512, :])
        beta = wp.T @ b_qkv[vcols]
        if g == 0:
            beta = beta + b_proj
        in_maps.append({
            "x_t": np.ascontiguousarray(x[b].T).astype(ml_dtypes.bfloat16),
            "w_qk": np.ascontiguousarray(w_qk).astype(ml_dtypes.bfloat16),
            "w_v": np.ascontiguousarray(w_qkv[:, vcols]).astype(ml_dtypes.bfloat16),
            "b_qk": b_qk.reshape(8, P, 1),
            "w_proj": wp.astype(ml_dtypes.bfloat16),
            "beta": beta.reshape(1, D).astype(ml_dtypes.bfloat16),
        })
    return in_maps


def kernel(x, w_qkv, b_qkv, w_proj, b_proj, trace=False, **run_kwargs):
    global LAST_RESULTS
    nc = _build()
    in_maps = make_in_maps(x, w_qkv, b_qkv, w_proj, b_proj)
    res = run_bass_kernel_spmd(
        nc, in_maps, core_ids=list(range(NCORES)), trace=trace, **run_kwargs
    )
    LAST_RESULTS = res
    out = np.empty((B, T, D), np.float32)
    for b in range(B):
        ev = np.asarray(res.results[2 * b]["out"], dtype=np.float32)
        od = np.asarray(res.results[2 * b + 1]["out"], dtype=np.float32)
        for c in range(8):
            out[b, c * 256 : c * 256 + 128] = ev[c * 128 : (c + 1) * 128]
            out[b, c * 256 + 128 : (c + 1) * 256] = od[c * 128 : (c + 1) * 128]
    return out


# revision 28
# speedup vs baseline: 1.3333x; 1.0609x over previous
"""Causal self-attention Bass kernel for 8 trn2 NeuronCores.

Problem: B=4, T=2048, D=1024, H=16 causal self-attention (qkv proj + attn + out proj).

Sharding: core c = 2*b + g handles batch b (=c//2) and head-group g (=c%2, 8 heads).

Structure (J-pipelined, chunked collective):
  - x is transposed + bf16-cast on the HOST: xT [D, T] arrives ready for matmuls.
  - V projection per t-quarter: vv[i] [keys=128, 4mp x (65A|65B)] bf16, ones col
    at slot 64 of each 65-block (softmax denominator via the AV matmul).
  - QK projection per T-quarter n: qkT[m] [128 dims, T] bf16 (+bias).
  - Attention per query block J (512 q) x head pair mp: transposed-score flash
    loop over key blocks j<=diag; exp on ACT -> at bf16; causal masks on DVE;
    AV accumulates [65, 512] psum (row 64 = denominator).
  - Normalization per (mp, J): reciprocal_approx_fast on the denominator row,
    partition_broadcast, muls; B-half is normalized pre-shift then DMA'd to
    partitions 64:128.
  - Output projection per J -> rs_in[J] bf16 [512, 1024]; pairwise
    ReduceScatter per J (overlaps attention of J+1); even core gets rows
    [0:256) of each 512-row chunk, odd gets [256:512).
Host reassembles interleaved 256-row chunks per batch.
"""

from contextlib import ExitStack

import ml_dtypes
import numpy as np

import concourse.bass as bass
import concourse.mybir as mybir
import concourse.tile as tile
from concourse import bacc
from concourse.bass_utils import run_bass_kernel_spmd

B, T, D, H = 4, 2048, 1024, 16
HD = D // H  # 64
NCORES = 8
P = 128
f32 = mybir.dt.float32
f32r = mybir.dt.float32r
bf16 = mybir.dt.bfloat16
EXP = mybir.ActivationFunctionType.Exp

_CACHE = {}
LAST_RESULTS = None
_DEBUG_SINK = None


def _dbg(nc, name, ap):
    if _DEBUG_SINK is not None and name in _DEBUG_SINK:
        nc.sync.dma_start(_DEBUG_SINK[name].ap(), ap)


def _emit(nc, tc, xt_d, wqk_d, wv_d, bqk_d, wproj_d, beta_d, out_d):
    with ExitStack() as ctx:
        # ---------------- constants / persistent tiles ----------------
        const = ctx.enter_context(tc.tile_pool(name="const", bufs=1))
        mask_tri = const.tile([P, P], bf16, tag="mask_tri")
        nc.gpsimd.memset(mask_tri[:], 1.0)
        nc.gpsimd.affine_select(
            out=mask_tri[:], in_=mask_tri[:],
            compare_op=mybir.AluOpType.is_ge, fill=0.0,
            base=0, pattern=[[1, P]], channel_multiplier=-1,
        )
        zeros384 = const.tile([P, 384], bf16, tag="zeros384")
        nc.vector.memset(zeros384[:], 0.0)
        bq = [const.tile([P, 1], f32, tag=f"bq{m}", name=f"bq{m}") for m in range(8)]
        beta_b = const.tile([P, D], bf16, tag="beta_b")

        # persistent activations
        xt_pool = ctx.enter_context(tc.tile_pool(name="xt", bufs=1))
        xT = [xt_pool.tile([P, T], bf16, tag=f"xT{k}", name=f"xT{k}") for k in range(8)]
        qkt_pool = ctx.enter_context(tc.tile_pool(name="qkt", bufs=1))
        qkT = [qkt_pool.tile([P, T], bf16, tag=f"qkT{m}", name=f"qkT{m}") for m in range(8)]
        vv_pool = ctx.enter_context(tc.tile_pool(name="vv", bufs=1))
        vv = [vv_pool.tile([P, 520], bf16, tag=f"vv{i}", name=f"vv{i}") for i in range(16)]

        # weights
        wp = ctx.enter_context(tc.tile_pool(name="wts", bufs=1))
        wqk_t = [wp.tile([P, 1024], bf16, tag=f"wqk{k}", name=f"wqk{k}") for k in range(8)]
        wv_t = [wp.tile([P, 512], bf16, tag=f"wv{k}", name=f"wv{k}") for k in range(8)]
        wproj_t = [wp.tile([P, D], bf16, tag=f"wp{hp}", name=f"wp{hp}") for hp in range(4)]

        # ones columns of vv (denominator trick): col 64 of each 65-block
        ones8 = const.tile([P, 8], bf16, tag="ones8")
        nc.vector.memset(ones8[:], 1.0)
        ones_src = ones8[:].rearrange("p (mp h one) -> p mp h one", mp=4, h=2)
        for i in range(16):
            dst = vv[i][:].rearrange("p (mp h d) -> p mp h d", mp=4, h=2)
            nc.vector.tensor_copy(dst[:, :, :, 64:65], ones_src[:, :, :, :])

        # working pools
        onp = ctx.enter_context(tc.tile_pool(name="outn", bufs=3))
        atp = ctx.enter_context(tc.tile_pool(name="atp", bufs=4))
        nrm = ctx.enter_context(tc.tile_pool(name="nrm", bufs=2))
        finp = ctx.enter_context(tc.tile_pool(name="finp", bufs=3))

        spp = ctx.enter_context(tc.tile_pool(name="spp", bufs=2, space="PSUM"))
        stps = ctx.enter_context(tc.tile_pool(name="stps", bufs=2, space="PSUM"))
        oups = ctx.enter_context(tc.tile_pool(name="oups", bufs=1, space="PSUM"))

        dram = ctx.enter_context(tc.tile_pool(name="dram", bufs=1, space="DRAM"))
        # one RS chunk per 256-row half of each query block J (8 total):
        # separate tiles so each collective depends only on its own writes
        rs_in = [dram.tile([256, D], bf16, tag=f"rsi{c}", name=f"rsi{c}") for c in range(8)]
        rs_out = [dram.tile([128, D], bf16, tag=f"rso{c}", name=f"rso{c}") for c in range(8)]

        # ~4us of dummy matmuls at the head of the PE queue: flips the HAM
        # clock gate to 8/8 while the first input DMAs are still in flight,
        # so the real matmuls start at 2.4 GHz.
        wps = spp.tile([P, 384], f32, tag="sp", name="warm")
        for d in range(24):
            nc.tensor.matmul(
                wps[:], zeros384[:, 0:128], zeros384[:],
                start=(d == 0), stop=(d == 23),
            )

        # ---------------- DMAs: quarter 0 + weights first, 3 queues ----------------
        for k in range(8):
            nc.sync.dma_start(xT[k][:, 0:512], xt_d.ap()[k * P : (k + 1) * P, 0:512])
        for k in range(8):
            nc.scalar.dma_start(wv_t[k][:], wv_d.ap()[k * P : (k + 1) * P, :])
        for k in range(8):
            nc.gpsimd.dma_start(wqk_t[k][:], wqk_d.ap()[k * P : (k + 1) * P, :])
        for m in range(8):
            nc.gpsimd.dma_start(bq[m][:], bqk_d.ap()[m])
        for q in range(1, 4):
            for k in range(8):
                nc.sync.dma_start(
                    xT[k][:, q * 512 : (q + 1) * 512],
                    xt_d.ap()[k * P : (k + 1) * P, q * 512 : (q + 1) * 512],
                )
        for hp in range(4):
            nc.gpsimd.dma_start(wproj_t[hp][:], wproj_d.ap()[hp * P : (hp + 1) * P, :])
        nc.gpsimd.dma_start(beta_b[0:1, :], beta_d.ap())
        nc.gpsimd.partition_broadcast(beta_b[:], beta_b[0:1, :], channels=P)

        def vproj(q, ils=range(4)):
            # v for t-tiles of quarter q: vv[i] [keys=128, (mp h 65)]
            for il in ils:
                i = q * 4 + il
                ps = spp.tile([P, 512], f32, tag="sp", name=f"vp{i}")
                for k in range(8):
                    nc.tensor.matmul(
                        ps[:],
                        xT[k][:, i * P : (i + 1) * P],
                        wv_t[k][:],
                        start=(k == 0), stop=(k == 7),
                    )
                src = ps[:].rearrange("p (mp h d) -> p mp h d", mp=4, h=2)
                dst = vv[i][:].rearrange("p (mp h d) -> p mp h d", mp=4, h=2)
                nc.vector.tensor_copy(dst[:, :, :, 0:64], src[:, :, :, :])

        def qkproj(n, ms=range(8)):
            ns = slice(n * 512, (n + 1) * 512)
            for m in ms:
                ps = spp.tile([P, 512], f32, tag="sp", name=f"qkp{m}n{n}")
                for k in range(8):
                    nc.tensor.matmul(
                        ps[:],
                        wqk_t[k][:, m * P : (m + 1) * P],
                        xT[k][:, ns],
                        start=(k == 0), stop=(k == 7),
                    )
                nc.vector.tensor_scalar_add(qkT[m][:, ns], ps[:], bq[m][:])

        def attn(J, fillers=()):
            Js = slice(J * 512, (J + 1) * 512)
            nj = 4 * J + 4
            fillers = list(fillers)
            outN = [onp.tile([P, 512], bf16, tag=f"outN{mp}", name=f"outN{mp}J{J}")
                    for mp in range(4)]
            for mp in range(4):
                qs, ks = qkT[mp], qkT[4 + mp]
                ouA = oups.tile([65, 512], f32, tag="ouA")
                ouB = oups.tile([65, 512], f32, tag="ouB")
                for j in range(nj):
                    sT = stps.tile([P, 1024], f32, tag="sT")
                    js = slice(j * P, (j + 1) * P)
                    i = j - 4 * J
                    # diagonal-straddling blocks: queries < c0 are fully
                    # masked, so scores/exp/AV all restrict to [c0:512)
                    c0 = 128 * i if i > 0 else 0
                    qJs = slice(J * 512 + c0, (J + 1) * 512)
                    w = 512 - c0
                    nc.tensor.matmul(
                        sT[:, 0:w], ks[0:64, js], qs[0:64, qJs],
                        start=True, stop=True, tile_position=(0, 0),
                    )
                    nc.tensor.matmul(
                        sT[:, 512 : 512 + w], ks[64:128, js], qs[64:128, qJs],
                        start=True, stop=True, tile_position=(64, 0),
                    )
                    at = atp.tile([P, 1024], bf16, tag="at")
                    src_v = sT[:].rearrange("p (h c) -> p h c", h=2)
                    dst_v = at[:].rearrange("p (h c) -> p h c", h=2)
                    nc.scalar.activation(
                        dst_v[:, :, 0:w], src_v[:, :, 0:w],
                        EXP, bias=0.0, scale=0.125,
                    )
                    if i >= 0:
                        # triangle mask on the first live 128 columns
                        for h0 in (0, 512):
                            nc.vector.tensor_mul(
                                at[:, h0 : h0 + 128],
                                at[:, h0 : h0 + 128], mask_tri[:],
                            )
                    if mp == 0 and J == 0 and j == 0:
                        _dbg(nc, "at000", at[:])
                    nc.tensor.matmul(
                        ouA[:, c0:512], vv[j][:, 130 * mp : 130 * mp + 65],
                        at[:, 0:w],
                        start=(j == 0), stop=(j == nj - 1),
                    )
                    nc.tensor.matmul(
                        ouB[:, c0:512], vv[j][:, 130 * mp + 65 : 130 * mp + 130],
                        at[:, 512 : 512 + w],
                        start=(j == 0), stop=(j == nj - 1),
                    )
                    if J == 0 and fillers:
                        fillers.pop(0)()
                # normalization: denominator (psum row 64) reciprocal,
                # broadcast, scale. B half normalized at partitions 0:64 then
                # DMA-shifted to outN partitions 64:128.
                dA = nrm.tile([1, 512], f32, tag="dA")
                dB = nrm.tile([1, 512], f32, tag="dB")
                nc.any.tensor_copy(dA[:], ouA[64:65, :])
                nc.any.tensor_copy(dB[:], ouB[64:65, :])
                dRA = nrm.tile([1, 512], f32, tag="dRA")
                dRB = nrm.tile([1, 512], f32, tag="dRB")
                nc.vector.reciprocal_approx_fast(dRA[:], dA[:])
                nc.vector.reciprocal_approx_fast(dRB[:], dB[:])
                cA = nrm.tile([1, 512], bf16, tag="cA")
                cB = nrm.tile([1, 512], bf16, tag="cB")
                nc.vector.tensor_copy(cA[:], dRA[:])
                nc.vector.tensor_copy(cB[:], dRB[:])
                bcA = nrm.tile([64, 512], bf16, tag="bcA")
                bcB = nrm.tile([P, 512], bf16, tag="bcB")
                nc.gpsimd.partition_broadcast(bcA[:, :], cA[:], channels=64)
                nc.gpsimd.partition_broadcast(bcB[:, :], cB[:], channels=P)
                # evict psum promptly (frees the AV banks for the next head
                # pair); B half is DMA'd raw to partitions 64:128 early and
                # normalized in place (keeps the shift DMA off the chain tail)
                tbA = nrm.tile([64, 512], bf16, tag="tbA")
                tbB = nrm.tile([64, 512], bf16, tag="tbB")
                nc.any.tensor_copy(tbA[:], ouA[0:64, :])
                nc.any.tensor_copy(tbB[:], ouB[0:64, :])
                nc.sync.dma_start(outN[mp][64:128, :], tbB[:])
                nc.vector.tensor_mul(outN[mp][0:64, :], tbA[:], bcA[:, :])
                nc.vector.tensor_mul(
                    outN[mp][64:128, :], outN[mp][64:128, :], bcB[64:128, :]
                )
                # interleave independent fill work (prev proj / next
                # projections) so the in-order PE queue never starves on the
                # normalization chain or exp latency
                if fillers:
                    fillers.pop(0)()
            while fillers:
                fillers.pop(0)()
            return outN

        def rs_chunk(c):
            if globals().get("_NO_COLLECTIVE"):
                nc.sync.dma_start(out_d.ap()[c * P : (c + 1) * P, :], rs_in[c][0:P, :])
            else:
                nc.gpsimd.collective_compute(
                    "ReduceScatter", mybir.AluOpType.add,
                    replica_groups=[[0, 1], [2, 3], [4, 5], [6, 7]],
                    ins=[rs_in[c].opt()], outs=[rs_out[c].opt()],
                )
                nc.sync.dma_start(out_d.ap()[c * P : (c + 1) * P, :], rs_out[c][:])

        def proj(J, outN):
            for i2 in range(4):
                c = 2 * J + i2 // 2  # 256-row RS chunk index
                for n in range(2):
                    ps = spp.tile([P, 512], f32, tag="sp", name=f"fp{J}i{i2}n{n}")
                    for hp in range(4):
                        nc.tensor.matmul(
                            ps[:],
                            outN[hp][:, i2 * P : (i2 + 1) * P],
                            wproj_t[hp][:, n * 512 : (n + 1) * 512],
                            start=(hp == 0), stop=(hp == 3),
                        )
                    fin = finp.tile([P, 512], bf16, tag="fin")
                    nc.vector.tensor_add(fin[:], ps[:], beta_b[:, n * 512 : (n + 1) * 512])
                    nc.sync.dma_start(
                        rs_in[c][(i2 % 2) * P : (i2 % 2 + 1) * P, n * 512 : (n + 1) * 512],
                        fin[:],
                    )
                if i2 % 2 == 1:
                    rs_chunk(c)

        # ---------------- main pipeline ----------------
        vproj(0)
        qkproj(0)
        pending = None
        for J in range(4):
            fillers = []
            if pending is not None:
                Jp, outNp = pending
                fillers.append(lambda Jp=Jp, o=outNp: proj(Jp, o))
            if J == 0:
                # fine-grained fillers: consumed per key-block in attn(0),
                # which is short and exp-latency-bound
                for il in range(4):
                    fillers.append(lambda il=il: vproj(1, [il]))
                for m in range(8):
                    fillers.append(lambda m=m: qkproj(1, [m]))
            elif J < 3:
                fillers.append(lambda q=J + 1: vproj(q))
                fillers.append(lambda q=J + 1: qkproj(q, range(4)))
                fillers.append(lambda q=J + 1: qkproj(q, range(4, 8)))
            outN = attn(J, fillers)
            pending = (J, outN)
        proj(pending[0], pending[1])
        _dbg(nc, "qkT0", qkT[0][:])
        _dbg(nc, "qkT4", qkT[4][:])
        _dbg(nc, "vv0", vv[0][:])
        _dbg(nc, "xT0", xT[0][:])


def _build():
    if "nc" in _CACHE:
        return _CACHE["nc"]
    nc = bacc.Bacc("TRN2", target_bir_lowering=False, debug=False, num_devices=NCORES)
    xt_d = nc.dram_tensor("x_t", [D, T], bf16, kind="ExternalInput")
    wqk_d = nc.dram_tensor("w_qk", [D, 1024], bf16, kind="ExternalInput")
    wv_d = nc.dram_tensor("w_v", [D, 512], bf16, kind="ExternalInput")
    bqk_d = nc.dram_tensor("b_qk", [8, P, 1], f32, kind="ExternalInput")
    wproj_d = nc.dram_tensor("w_proj", [512, D], bf16, kind="ExternalInput")
    beta_d = nc.dram_tensor("beta", [1, D], bf16, kind="ExternalInput")
    out_d = nc.dram_tensor("out", [T // 2, D], bf16, kind="ExternalOutput")
    with tile.TileContext(nc) as tc:
        _emit(nc, tc, xt_d, wqk_d, wv_d, bqk_d, wproj_d, beta_d, out_d)
    nc.compile()
    _CACHE["nc"] = nc
    return nc


def make_in_maps(x, w_qkv, b_qkv, w_proj, b_proj):
    x = np.asarray(x, np.float32)
    w_qkv = np.asarray(w_qkv, np.float32)
    b_qkv = np.asarray(b_qkv, np.float32)
    w_proj = np.asarray(w_proj, np.float32)
    b_proj = np.asarray(b_proj, np.float32)
    in_maps = []
    for c in range(NCORES):
        b, g = c // 2, c % 2
        qcols = slice(g * 512, (g + 1) * 512)
        kcols = slice(D + g * 512, D + (g + 1) * 512)
        vcols = slice(2 * D + g * 512, 2 * D + (g + 1) * 512)
        w_qk = np.concatenate([w_qkv[:, qcols], w_qkv[:, kcols]], axis=1)
        b_qk = np.concatenate([b_qkv[qcols], b_qkv[kcols]])
        wp = np.ascontiguousarray(w_proj[g * 512 : (g + 1) * 512, :])
        beta = wp.T @ b_qkv[vcols]
        if g == 0:
            beta = beta + b_proj
        in_maps.append({
            "x_t": np.ascontiguousarray(x[b].T).astype(ml_dtypes.bfloat16),
            "w_qk": np.ascontiguousarray(w_qk).astype(ml_dtypes.bfloat16),
            "w_v": np.ascontiguousarray(w_qkv[:, vcols]).astype(ml_dtypes.bfloat16),
            "b_qk": b_qk.reshape(8, P, 1),
            "w_proj": wp.astype(ml_dtypes.bfloat16),
            "beta": beta.reshape(1, D).astype(ml_dtypes.bfloat16),
        })
    return in_maps


def kernel(x, w_qkv, b_qkv, w_proj, b_proj, trace=False, **run_kwargs):
    global LAST_RESULTS
    nc = _build()
    in_maps = make_in_maps(x, w_qkv, b_qkv, w_proj, b_proj)
    res = run_bass_kernel_spmd(
        nc, in_maps, core_ids=list(range(NCORES)), trace=trace, **run_kwargs
    )
    LAST_RESULTS = res
    out = np.empty((B, T, D), np.float32)
    for b in range(B):
        ev = np.asarray(res.results[2 * b]["out"], dtype=np.float32)
        od = np.asarray(res.results[2 * b + 1]["out"], dtype=np.float32)
        for c in range(8):
            out[b, c * 256 : c * 256 + 128] = ev[c * 128 : (c + 1) * 128]
            out[b, c * 256 + 128 : (c + 1) * 256] = od[c * 128 : (c + 1) * 128]
    return out
